# revision 4
# baseline (speedup 1.0000x reference)
"""Calibrated cross-entropy 2D (histogram binning) — Trainium2 Bass kernel.

Problem: nn_CalibratedCE2d_88493506167215
  predict    [8, 21, 513, 513] f32   (NCHW logits)
  target     [8, 513, 513]     int   (class ids)
  confidence [2105352]         f32
  accuracies [15]              f32
  n_bin      15

  loss = -sum_i w_i * logp_target_i / size
  where w_i = coeff[bin(confidence_i)] if selected else 0,
        coeff_b = acc_b*10 - (1-acc_b)*50 (only coeff>0 bins selected),
        size = number of selected pixels.

Sharding: data-parallel over the batch axis — one image (n) per NeuronCore,
8 cores.  Per-core device program (pixel-major [128, F] tiles):
  for each class c in 0..20:
      load plane slice x_c, e_c = exp(x_c)                 (ACT, bf16 out)
      masked_c = (tgt == c) * e_c                          (DVE fused stt)
      PSUM A += I @ e_c ; PSUM B += I @ masked_c           (PE identity matmuls)
  A = sum_c exp(x_c) per pixel, B = exp(x_target) per pixel
  logp_t = ln(B) - ln(A)
  out partials: sum_f w*ln(B), sum_f w*ln(A)               (DVE stt + accum)
Host: per-pixel weights w from confidence (identical f32 arithmetic as the
reference), 8-way partial-sum combine, final divide.  The last pixel of each
image (263169 = 128*2056 + 1 does not tile evenly) is folded in on the host.
"""

import numpy as np
import ml_dtypes
from contextlib import ExitStack

N_IMG, C, H, W = 8, 21, 513, 513
PX = H * W                    # 263169 pixels per image
FD = 2048                     # main tile free dim
MAIN = 128 * FD               # 262144 pixels in the main grid
TFD = 8                       # tail tile free dim
LEFT = MAIN + 128 * TFD       # 263168; pixel LEFT..PX-1 handled on host (1 px)
N_TOTAL_BINS = 15

_NC_CACHE: dict = {}


def _build_program():
    import concourse.bass as bass
    import concourse.bacc as bacc
    import concourse.tile as tile
    from concourse import mybir

    f32 = mybir.dt.float32
    bf16 = mybir.dt.bfloat16
    Exp = mybir.ActivationFunctionType.Exp
    Ln = mybir.ActivationFunctionType.Ln
    is_equal = mybir.AluOpType.is_equal
    mult = mybir.AluOpType.mult
    bypass = mybir.AluOpType.bypass

    nc = bacc.Bacc(
        "TRN2",
        target_bir_lowering=False,
        debug=False,
        enable_asserts=False,
        num_devices=N_IMG,
    )
    x_d = nc.dram_tensor("x", [C, PX], f32, kind="ExternalInput")
    tgt_d = nc.dram_tensor("tgt", [PX], bf16, kind="ExternalInput")
    w_d = nc.dram_tensor("w", [PX], f32, kind="ExternalInput")
    id_d = nc.dram_tensor("ident", [128, 128], bf16, kind="ExternalInput")
    out_d = nc.dram_tensor("out", [128, 4], f32, kind="ExternalOutput")

    x = x_d.ap()
    tgt = tgt_d.ap()
    w = w_d.ap()

    with tile.TileContext(nc) as tc, ExitStack() as ctx:
        const_pool = ctx.enter_context(tc.tile_pool(name="const", bufs=1))
        xpool = ctx.enter_context(tc.tile_pool(name="xp", bufs=4))
        epool = ctx.enter_context(tc.tile_pool(name="ep", bufs=4))
        mpool = ctx.enter_context(tc.tile_pool(name="mp", bufs=4))
        tailpool = ctx.enter_context(tc.tile_pool(name="tails", bufs=1))
        postpool = ctx.enter_context(tc.tile_pool(name="post", bufs=1))
        psum_main = ExitStack()
        psum = psum_main.enter_context(tc.tile_pool(name="ps", bufs=1, space="PSUM"))

        idt = const_pool.tile([128, 128], bf16, tag="idt", name="idt")
        nc.sync.dma_start(idt[:], id_d.ap())
        tgt_m = const_pool.tile([128, FD], bf16, tag="tgtm", name="tgt_m")
        nc.sync.dma_start(tgt_m[:], tgt[0:MAIN].rearrange("(p f) -> p f", p=128))
        tgt_t = const_pool.tile([128, TFD], bf16, tag="tgtt", name="tgt_t")
        nc.sync.dma_start(tgt_t[:], tgt[MAIN:LEFT].rearrange("(p f) -> p f", p=128))
        w_m = const_pool.tile([128, FD], f32, tag="wm", name="w_m")
        nc.sync.dma_start(w_m[:], w[0:MAIN].rearrange("(p f) -> p f", p=128))
        w_t = const_pool.tile([128, TFD], f32, tag="wt", name="w_t")
        nc.sync.dma_start(w_t[:], w[MAIN:LEFT].rearrange("(p f) -> p f", p=128))

        A = psum.tile([128, FD], f32, tag="A", name="A")
        B = psum.tile([128, FD], f32, tag="B", name="B")

        e_tails = []
        m_tails = []
        for c in range(C):
            xm = xpool.tile([128, FD], f32, tag="xm", name=f"xm{c}")
            nc.sync.dma_start(
                xm[:], x[c : c + 1, 0:MAIN].rearrange("o (p f) -> (o p) f", p=128)
            )
            xt = xpool.tile([128, TFD], f32, tag="xt", name=f"xt{c}")
            nc.sync.dma_start(
                xt[:], x[c : c + 1, MAIN:LEFT].rearrange("o (p f) -> (o p) f", p=128)
            )
            em = epool.tile([128, FD], bf16, tag="em", name=f"em{c}")
            nc.scalar.activation(em[:], xm[:], Exp)
            et = tailpool.tile([128, TFD], bf16, tag=f"et{c}", name=f"et{c}")
            nc.scalar.activation(et[:], xt[:], Exp)
            mm = mpool.tile([128, FD], bf16, tag="mm", name=f"mm{c}")
            nc.vector.scalar_tensor_tensor(
                mm[:], tgt_m[:], float(c), em[:], op0=is_equal, op1=mult
            )
            mt = tailpool.tile([128, TFD], bf16, tag=f"mt{c}", name=f"mt{c}")
            nc.vector.scalar_tensor_tensor(
                mt[:], tgt_t[:], float(c), et[:], op0=is_equal, op1=mult
            )
            e_tails.append(et)
            m_tails.append(mt)
            for j in range(FD // 512):
                sl = slice(j * 512, (j + 1) * 512)
                nc.tensor.matmul(
                    A[:, sl], idt[:], em[:, sl], start=(c == 0), stop=(c == C - 1)
                )
                nc.tensor.matmul(
                    B[:, sl], idt[:], mm[:, sl], start=(c == 0), stop=(c == C - 1)
                )

        # ---- main post: logp_t = ln(B) - ln(A); accumulate w-weighted sums
        lb = postpool.tile([128, FD], f32, tag="lb", name="lb")
        la = postpool.tile([128, FD], f32, tag="la", name="la")
        nc.scalar.activation(lb[:], B[:], Ln)
        nc.scalar.activation(la[:], A[:], Ln)
        acc = postpool.tile([128, 4], f32, tag="acc", name="acc")
        scr = postpool.tile([128, FD], f32, tag="scr", name="scr")
        nc.vector.scalar_tensor_tensor(
            scr[:], lb[:], 0.0, w_m[:], op0=bypass, op1=mult, accum_out=acc[:, 0:1]
        )
        nc.vector.scalar_tensor_tensor(
            scr[:], la[:], 0.0, w_m[:], op0=bypass, op1=mult, accum_out=acc[:, 1:2]
        )

        # ---- tail region (pixels MAIN..LEFT), PSUM banks reused after main post
        psum_main.close()  # release A/B banks before allocating the tail pool
        psum2 = ctx.enter_context(tc.tile_pool(name="ps2", bufs=1, space="PSUM"))
        At = psum2.tile([128, TFD], f32, tag="At", name="At")
        Bt = psum2.tile([128, TFD], f32, tag="Bt", name="Bt")
        for c in range(C):
            nc.tensor.matmul(
                At[:], idt[:], e_tails[c][:], start=(c == 0), stop=(c == C - 1)
            )
            nc.tensor.matmul(
                Bt[:], idt[:], m_tails[c][:], start=(c == 0), stop=(c == C - 1)
            )
        lbt = postpool.tile([128, TFD], f32, tag="lbt", name="lbt")
        lat = postpool.tile([128, TFD], f32, tag="lat", name="lat")
        nc.scalar.activation(lbt[:], Bt[:], Ln)
        nc.scalar.activation(lat[:], At[:], Ln)
        scrt = postpool.tile([128, TFD], f32, tag="scrt", name="scrt")
        nc.vector.scalar_tensor_tensor(
            scrt[:], lbt[:], 0.0, w_t[:], op0=bypass, op1=mult, accum_out=acc[:, 2:3]
        )
        nc.vector.scalar_tensor_tensor(
            scrt[:], lat[:], 0.0, w_t[:], op0=bypass, op1=mult, accum_out=acc[:, 3:4]
        )

        nc.sync.dma_start(out_d.ap(), acc[:])

    nc.compile()
    return nc


def _get_nc():
    if "nc" not in _NC_CACHE:
        _NC_CACHE["nc"] = _build_program()
    return _NC_CACHE["nc"]


def _pixel_weights(conf: np.ndarray, accuracies: np.ndarray, n_bin: int):
    """Per-pixel weights, f32 arithmetic identical to the reference."""
    acc = np.asarray(accuracies, dtype=np.float32)[:n_bin]
    coeff = acc * np.float32(10.0) - (np.float32(1.0) - acc) * np.float32(50.0)
    wtab = np.where(coeff > np.float32(0.0), coeff, np.float32(0.0)).astype(np.float32)
    # table16[k] for k = ceil(conf*15) in 0..15; k=0 (conf==0) -> invalid -> 0
    table16 = np.concatenate([[np.float32(0.0)], wtab]).astype(np.float32)
    t15 = conf * np.float32(N_TOTAL_BINS)          # same f32 product as reference
    k16 = np.ceil(t15).astype(np.int32)
    k16 = np.clip(k16, 0, n_bin)
    wfull = table16[k16]
    valid = (conf > np.float32(0.0)) & (conf <= np.float32(1.0))
    wfull = np.where(valid, wfull, np.float32(0.0)).astype(np.float32)
    return wfull


def _prepare(predict, target, confidence, accuracies, n_bin):
    predict = np.ascontiguousarray(np.asarray(predict, dtype=np.float32))
    target = np.asarray(target)
    conf = np.asarray(confidence, dtype=np.float32)
    accuracies = np.asarray(accuracies, dtype=np.float32)
    n_bin = int(n_bin)
    assert predict.shape == (N_IMG, C, H, W) and n_bin == N_TOTAL_BINS

    wfull = _pixel_weights(conf, accuracies, n_bin)
    size = float(np.count_nonzero(wfull))

    xs = predict.reshape(N_IMG, C, PX)
    tg = target.reshape(N_IMG, PX).astype(np.int64)
    wf = wfull.reshape(N_IMG, PX)
    ident = np.eye(128, dtype=ml_dtypes.bfloat16)

    in_maps = [
        {
            "x": xs[n],
            "tgt": tg[n].astype(ml_dtypes.bfloat16),
            "w": wf[n],
            "ident": ident,
        }
        for n in range(N_IMG)
    ]
    return xs, tg, wf, size, in_maps


def _combine(res_list, xs, tg, wf, size) -> np.ndarray:
    S = 0.0
    for n in range(N_IMG):
        o = np.asarray(res_list[n]["out"], dtype=np.float64)
        S += (o[:, 0].sum() - o[:, 1].sum()) + (o[:, 2].sum() - o[:, 3].sum())

    # host-side leftover pixels (one per image: index LEFT..PX-1)
    for n in range(N_IMG):
        for p in range(LEFT, PX):
            xv = xs[n][:, p].astype(np.float64)
            m = xv.max()
            lse = np.log(np.exp(xv - m).sum()) + m
            xt = xv[tg[n][p]]
            S += float(wf[n][p]) * (xt - lse)

    loss = np.float32(-(S / size))
    return np.asarray(loss, dtype=np.float32)


def run_device(in_maps, trace=False, **kwargs):
    from concourse.bass_utils import run_bass_kernel_spmd

    nc = _get_nc()
    return run_bass_kernel_spmd(
        nc, in_maps, core_ids=list(range(N_IMG)), trace=trace, **kwargs
    )


def kernel(predict, target, confidence, accuracies, n_bin) -> np.ndarray:
    xs, tg, wf, size, in_maps = _prepare(predict, target, confidence, accuracies, n_bin)
    res = run_device(in_maps)
    return _combine(res.results, xs, tg, wf, size)
